# revision 1
# baseline (speedup 1.0000x reference)
"""Causal self-attention (B=1, S=4096, D=768, H=12, dh=64) on 8 TRN2 NeuronCores.

Strategy:
  - Sequence-parallel QKV projections + RoPE (each core projects 512 rows).
  - K/V (bf16; V carries a ones-column for the softmax denominator) are
    projected, rope'd and bounced out one 128-row quarter at a time, each
    quarter feeding its own AllGather so gathers overlap later projection
    work; the Q projection is emitted last to overlap the gathers too.
  - Attention is query-sharded with a stride-8 interleave (core c owns query
    rows c::8) so causal work is balanced and the program is SPMD-uniform;
    all per-core variation is input data (x slices, rope tables, masks).
  - KV rows are owned in interleaved 64-row blocks (block b -> core b%8) so the
    gather splits into 4 pipelined quarter-AllGathers, each delivering key
    chunks 8u..8u+7 in causal consumption order; gather + K/V reload overlap
    attention on the early chunks.
  - Transposed layout throughout: S^T = K^T.T @ Q^T has keys on partitions, so
    the softmax sum falls out of the AV matmul via the ones-row of V.
  - QK runs two heads concurrently via PE row-groups (0,*)/(64,*) with separate
    PSUM banks (3-chunk groups: 2x3 st banks + 2 ot accumulators = 8 banks).
  - Causal mask = per-128-key-chunk band multiply on a [128,3,48] window
    (band position within a group is core-independent; mask values are data).
  - RoPE: rot = A*cos + swap(A)*sin_signed with the sign in the host-built
    sin table; for K the swap is 4 quadrant-aligned DVE copies straight out
    of PSUM (keeps the AllGather-feeding chain short), for Q it is SBUF DMAs
    off a Scalar-engine PSUM copy. Logit scale is folded into exp().
"""

import numpy as np
import ml_dtypes

import concourse.bass as bass
import concourse.bacc as bacc
import concourse.tile as tile
import concourse.mybir as mybir
import concourse.bass_utils as bass_utils

NCORES = 8
S = 4096
D = 768
H = 12
DH = 64
HALF = 32
P = 128
SL = S // NCORES          # 512 local queries / kv rows per core
KSUB = D // P             # 6
NKC = S // P              # 32 key chunks of 128
GK = 4                    # key chunks per exp group
NG = NKC // GK            # 8 groups
KS = D * SL               # K^T slice elems (768*512)
VW = H * (DH + 1)         # 780: V row width incl. ones col per head
VS = SL * VW              # V slice elems
NQ = 4                    # pipelined AllGather quarters
KQ = D * P                # K^T part per quarter (768*128)
VQ = P * VW               # V part per quarter
RQ = KQ + VQ              # per-rank elems per quarter
F32 = mybir.dt.float32
BF16 = mybir.dt.bfloat16

_cache = {}


def _build(repeats=1, fake_gather=False, stop_after=None):
    nc = bacc.Bacc(
        "TRN2",
        target_bir_lowering=False,
        debug=False,
        enable_asserts=False,
        num_devices=1 if fake_gather else NCORES,
    )
    inp = {}
    for name, shape, dt in [
        ("xq", [D, SL], BF16),
        ("xkv", [D, SL], BF16),
        ("cosq", [P, SL], BF16),
        ("sinq", [P, SL], BF16),
        ("cosk", [P, SL], BF16),
        ("sink", [P, SL], BF16),
        ("mask3", [P, 3, 48], BF16),
        ("wq", [D, D], BF16),
        ("wk", [D, D], BF16),
        ("wv", [D, D], BF16),
        ("wo", [D, D], BF16),
    ]:
        inp[name] = nc.dram_tensor(name, shape, dt, kind="ExternalInput")
    out_d = nc.dram_tensor("out", [KSUB, P, SL], F32, kind="ExternalOutput")

    with tile.TileContext(nc) as tc:
      for _rep in range(repeats):
        with (
            tc.tile_pool(name="persist", bufs=1) as persist,
            tc.tile_pool(name="dram", bufs=1, space="DRAM") as dram,
        ):
            # ---- persistent tiles ----
            qrot_t = [
                persist.tile([P, SL], BF16, name=f"qrot{s_}", tag=f"qrot{s_}")
                for s_ in range(KSUB)
            ]
            osb = persist.tile([64, H, SL], BF16)
            mask_sb = persist.tile([P, 3, 48], BF16)
            nc.sync.dma_start(mask_sb[:], inp["mask3"].ap())
            wo_sb = persist.tile([64, H, D], BF16)
            for h in range(H):
                nc.sync.dma_start(
                    wo_sb[:, h, :],
                    inp["wo"].ap().rearrange("(h p) e -> p h e", p=64)[:, h, :],
                )

            kvin = dram.tile([NQ, RQ], BF16)
            kvout = [
                dram.tile(
                    [NCORES, RQ],
                    BF16,
                    name=f"kvout{u}",
                    addr_space="Local" if fake_gather else "Shared",
                )
                for u in range(NQ)
            ]

            # ================= Phase A: projections + rope =================
            with (
                tc.tile_pool(name="pw", bufs=1) as pw,
                tc.tile_pool(name="px", bufs=1) as px,
                tc.tile_pool(name="pt", bufs=3) as pt,
                tc.tile_pool(name="psA", bufs=2, space="PSUM") as psA,
            ):
                w_sb = {}
                for name in ["wq", "wk", "wv"]:
                    w_sb[name] = pw.tile([P, KSUB, D], BF16, name=f"{name}_sb")
                    for ks in range(KSUB):
                        nc.sync.dma_start(
                            w_sb[name][:, ks, :],
                            inp[name].ap().rearrange("(ks p) m -> p ks m", p=P)[
                                :, ks, :
                            ],
                        )
                xq_sb = px.tile([P, KSUB, SL], BF16)
                xkv_sb = px.tile([P, KSUB, SL], BF16)
                for ks in range(KSUB):
                    nc.sync.dma_start(
                        xq_sb[:, ks, :],
                        inp["xq"].ap().rearrange("(ks p) n -> p ks n", p=P)[:, ks, :],
                    )
                    nc.sync.dma_start(
                        xkv_sb[:, ks, :],
                        inp["xkv"].ap().rearrange("(ks p) n -> p ks n", p=P)[:, ks, :],
                    )
                trig = {}
                for name in ["cosq", "sinq", "cosk", "sink"]:
                    trig[name] = px.tile([P, SL], BF16, name=f"{name}_sb")
                    nc.sync.dma_start(trig[name][:], inp[name].ap())

                vloc = px.tile([P, S // P // NCORES, H, DH + 1], BF16)

                def project_rope(wname, x_sb, cos_t, sin_t, dest, subs):
                    # sin_t is block-signed: rows 0-31=-sin, 32-63=+sin, etc.
                    for s in subs:
                        pa = psA.tile([P, SL], F32, name="pa", tag="pa")
                        for ks in range(KSUB):
                            nc.tensor.matmul(
                                pa[:],
                                lhsT=w_sb[wname][:, ks, s * P : (s + 1) * P],
                                rhs=x_sb[:, ks, :],
                                start=(ks == 0),
                                stop=(ks == KSUB - 1),
                            )
                        pab = pt.tile([P, SL], BF16, name="pab", tag="pab")
                        nc.scalar.copy(pab[:], pa[:])
                        swp = pt.tile([P, SL], BF16, name="swp", tag="swp")
                        for (dd, ss2) in [(0, 32), (32, 0), (64, 96), (96, 64)]:
                            nc.sync.dma_start(
                                swp[dd : dd + 32, :], pab[ss2 : ss2 + 32, :]
                            )
                        t1 = pt.tile([P, SL], BF16, name="t1", tag="t1")
                        t2 = pt.tile([P, SL], BF16, name="t2", tag="t2")
                        nc.vector.tensor_mul(t1[:], pab[:], cos_t[:])
                        nc.vector.tensor_mul(t2[:], swp[:], sin_t[:])
                        nc.vector.tensor_add(dest[s][:], t1[:], t2[:])

                # K + V projection, rope and bounce-out one QUARTER (128 kv
                # rows) at a time so each quarter-AllGather launches as soon
                # as its data exists, overlapping later projection work and
                # attention on early key chunks.
                nc.vector.memset(vloc[:, :, :, DH : DH + 1], 1.0)
                for u in range(NQ):
                    kq = pt.tile([P, KSUB, P], BF16, name="kq", tag="kq")
                    for s in range(KSUB):
                        pa = psA.tile([P, P], F32, name="pak", tag="pak")
                        for ks in range(KSUB):
                            nc.tensor.matmul(
                                pa[:],
                                lhsT=w_sb["wk"][:, ks, s * P : (s + 1) * P],
                                rhs=xkv_sb[:, ks, u * P : (u + 1) * P],
                                start=(ks == 0),
                                stop=(ks == KSUB - 1),
                            )
                        swp = pt.tile([P, P], BF16, name="swpk", tag="swpk")
                        for (dd, ss2) in [(0, 32), (32, 0), (64, 96), (96, 64)]:
                            nc.vector.tensor_copy(
                                swp[dd : dd + 32, :], pa[ss2 : ss2 + 32, :]
                            )
                        t1 = pt.tile([P, P], BF16, name="t1k", tag="t1k")
                        t2 = pt.tile([P, P], BF16, name="t2k", tag="t2k")
                        nc.vector.tensor_mul(
                            t1[:], pa[:], trig["cosk"][:, u * P : (u + 1) * P]
                        )
                        nc.vector.tensor_mul(
                            t2[:], swp[:], trig["sink"][:, u * P : (u + 1) * P]
                        )
                        nc.vector.tensor_add(kq[:, s, :], t1[:], t2[:])
                    pv = psA.tile([P, 2, SL], F32, name="pv", tag="pv")
                    for j in range(2):
                        for ks in range(KSUB):
                            nc.tensor.matmul(
                                pv[:, j, 0 : D // 2],
                                lhsT=xkv_sb[:, ks, u * P : (u + 1) * P],
                                rhs=w_sb["wv"][:, ks, j * (D // 2) : (j + 1) * (D // 2)],
                                start=(ks == 0),
                                stop=(ks == KSUB - 1),
                            )
                    for j in range(2):
                        nc.scalar.copy(
                            vloc[:, u, j * 6 : (j + 1) * 6, 0:DH],
                            pv[:, j, 0 : D // 2].rearrange("p (h d) -> p h d", d=DH),
                        )
                    nc.sync.dma_start(
                        kvin[u, 0:KQ].rearrange("(ks p n) -> p ks n", p=P, ks=KSUB),
                        kq[:],
                    )
                    nc.sync.dma_start(
                        kvin[u, KQ:].rearrange("(p h d) -> p h d", p=P, h=H),
                        vloc[:, u, :, :],
                    )
                    if fake_gather:
                        for c in range(NCORES):
                            nc.sync.dma_start(kvout[u][c], kvin[u])
                    else:
                        nc.gpsimd.collective_compute(
                            "AllGather",
                            mybir.AluOpType.bypass,
                            replica_groups=[list(range(NCORES))],
                            ins=[kvin[u].opt()],
                            outs=[kvout[u][:].opt()],
                        )
                    if u == 0:
                        # early Q subtiles: head-pair 0 can start its
                        # (ACT-bound) attention under the remaining
                        # (PE-bound) K-quarter projections
                        project_rope(
                            "wq", xq_sb, trig["cosq"], trig["sinq"], qrot_t, [0, 1]
                        )

                project_rope(
                    "wq", xq_sb, trig["cosq"], trig["sinq"], qrot_t, range(2, KSUB)
                )

            # ================= Phase B: attention =================
            if stop_after == "A":
                continue
            with (
                tc.tile_pool(name="pkv", bufs=1) as pkv,
                tc.tile_pool(name="pe", bufs=4) as pe,
                tc.tile_pool(name="pn", bufs=3) as pn,
                tc.tile_pool(name="psS", bufs=1, space="PSUM") as psS,
                tc.tile_pool(name="psO", bufs=1, space="PSUM") as psO,
            ):
                ksb_q, vsb_q = [], []
                for u in range(NQ):
                    kt = pkv.tile(
                        [P, KSUB, NCORES, P], BF16, name=f"ksbq{u}", tag=f"ksbq{u}"
                    )
                    for c in range(NCORES):
                        src = kvout[u][c, 0:KQ].rearrange(
                            "(ks p n) -> p ks n", p=P, ks=KSUB
                        )
                        # core c's cols 0:64 = global block 16u+c -> chunk slot
                        # c//2 half c%2; cols 64:128 = block 16u+8+c -> slot
                        # 4+c//2 half c%2. Chunk-contiguous keys for LDWEIGHTS.
                        po = 64 * (c % 2)
                        nc.sync.dma_start(
                            kt[:, :, c // 2, po : po + 64], src[:, :, 0:64]
                        )
                        nc.sync.dma_start(
                            kt[:, :, 4 + c // 2, po : po + 64], src[:, :, 64:128]
                        )
                    ksb_q.append(kt)
                    vt = pkv.tile(
                        [P, NCORES, H, DH + 1], BF16, name=f"vsbq{u}", tag=f"vsbq{u}"
                    )
                    for c in range(NCORES):
                        po = 64 * (c % 2)
                        nc.sync.dma_start(
                            vt[po : po + 64, c // 2, :, :],
                            kvout[u][c, KQ : KQ + 64 * VW].rearrange(
                                "(p h d) -> p h d", p=64, h=H
                            ),
                        )
                        nc.sync.dma_start(
                            vt[po : po + 64, 4 + c // 2, :, :],
                            kvout[u][c, KQ + 64 * VW :].rearrange(
                                "(p h d) -> p h d", p=64, h=H
                            ),
                        )
                    vsb_q.append(vt)

                NG3 = (NKC + 2) // 3  # 11 groups of <=3 chunks
                for hp in range(H // 2 if stop_after != "KV" else 0):
                    s = hp
                    ots = [
                        psO.tile([DH + 1, SL], F32, name=f"ot{j}", tag=f"ot{j}")
                        for j in range(2)
                    ]
                    for g in range(NG3):
                        chunks = range(3 * g, min(3 * g + 3, NKC))
                        nch = len(chunks)
                        xs = 48 * g
                        sts = [
                            psS.tile([P, 3, SL], F32, name=f"st{j}", tag=f"st{j}")
                            for j in range(2)
                        ]
                        # interleave the two heads' QK matmuls: row groups
                        # (0,*) and (64,*) run concurrently on the PE array
                        for i, kc in enumerate(chunks):
                            for j in range(2):
                                off = 64 * j
                                nc.tensor.matmul(
                                    sts[j][:, i, xs:SL],
                                    lhsT=ksb_q[kc // 8][off : off + 64, s, kc % 8, :],
                                    rhs=qrot_t[s][off : off + 64, xs:SL],
                                    start=True,
                                    stop=True,
                                )
                        expss = []
                        for j in range(2):
                            exps = pe.tile(
                                [P, 3, SL], BF16, name=f"exps{j}", tag=f"exps{j}"
                            )
                            nc.scalar.activation(
                                exps[:, 0:nch, xs:SL],
                                sts[j][:, 0:nch, xs:SL],
                                mybir.ActivationFunctionType.Exp,
                                scale=0.125,
                            )
                            mw = min(48, SL - xs)
                            nc.vector.tensor_mul(
                                exps[:, 0:nch, xs : xs + mw],
                                exps[:, 0:nch, xs : xs + mw],
                                mask_sb[:, 0:nch, 0:mw],
                            )
                            expss.append(exps)
                        for i, kc in enumerate(chunks):
                            for j in range(2):
                                nc.tensor.matmul(
                                    ots[j][:, xs:SL],
                                    lhsT=vsb_q[kc // 8][:, kc % 8, 2 * hp + j, :],
                                    rhs=expss[j][:, i, xs:SL],
                                    start=(kc == 0),
                                    stop=(kc == NKC - 1),
                                    skip_group_check=True,
                                )
                    for j in range(2):
                        h = 2 * hp + j
                        ot = ots[j]
                        # partition 64 is quadrant-aligned: the DVE can move
                        # the denominator row straight to partition 0 (probed
                        # on HW), shortening the normalize chain by a DMA hop.
                        den = pn.tile([1, SL], F32, name="den", tag="den")
                        nc.vector.tensor_copy(den[0:1, :], ot[64:65, :])
                        recip = pn.tile([1, SL], F32, name="recip", tag="recip")
                        nc.vector.reciprocal(recip[:], den[:])
                        recipb = pn.tile([64, SL], F32, name="recipb", tag="recipb")
                        nc.gpsimd.partition_broadcast(recipb[:], recip[:])
                        nc.vector.tensor_mul(osb[:, h, :], ot[0:64, :], recipb[:])

            # ================= Phase C: output projection =================
            if stop_after in ("B", "KV"):
                continue
            with (
                tc.tile_pool(name="pco", bufs=2) as pco,
                tc.tile_pool(name="psC", bufs=2, space="PSUM") as psC,
            ):
                for m in range(KSUB):
                    outp = psC.tile([P, SL], F32, name="outp", tag="outp")
                    for h in range(H):
                        nc.tensor.matmul(
                            outp[:],
                            lhsT=wo_sb[:, h, m * P : (m + 1) * P],
                            rhs=osb[:, h, :],
                            start=(h == 0),
                            stop=(h == H - 1),
                        )
                    ocp = pco.tile([P, SL], F32, name="ocp", tag="ocp")
                    nc.any.tensor_copy(ocp[:], outp[:])
                    nc.sync.dma_start(out_d.ap()[m], ocp[:])

    nc.compile()
    return nc


def _host_prep(x, position_ids, Wq, Wk, Wv, Wo):
    x2 = np.asarray(x, dtype=np.float32).reshape(S, D)
    pos = np.asarray(position_ids).reshape(S)

    fraction = (2.0 * np.arange(HALF, dtype=np.float32) / DH).astype(np.float32)
    timescale = (10000.0 ** fraction).astype(np.float32)  # [32]

    def tables(p_vec):
        sinu = (p_vec[None, :].astype(np.float32) / timescale[:, None]).astype(
            np.float32
        )
        cos = np.tile(np.cos(sinu).astype(np.float32), (4, 1))
        sin = np.sin(sinu).astype(np.float32)
        # signed for the swap formulation: first-half rows get -sin (they
        # subtract the swapped second half), second-half rows get +sin.
        sin = np.concatenate([-sin, sin, -sin, sin], axis=0)
        return cos.astype(ml_dtypes.bfloat16), sin.astype(ml_dtypes.bfloat16)

    bf = ml_dtypes.bfloat16
    weights = {
        "wq": np.ascontiguousarray(np.asarray(Wq, dtype=np.float32)).astype(bf),
        "wk": np.ascontiguousarray(np.asarray(Wk, dtype=np.float32)).astype(bf),
        "wv": np.ascontiguousarray(np.asarray(Wv, dtype=np.float32)).astype(bf),
        "wo": np.ascontiguousarray(np.asarray(Wo, dtype=np.float32)).astype(bf),
    }

    in_maps = []
    for c in range(NCORES):
        qrows = np.arange(SL) * NCORES + c
        # kv rows: 64-row blocks b with b % 8 == c, in ascending order
        kvrows = (
            (np.arange(NCORES) * NCORES + c)[:, None] * 64 + np.arange(64)[None, :]
        ).ravel()
        cosq, sinq = tables(pos[qrows])
        cosk, sink = tables(pos[kvrows])
        pp = np.arange(P)[:, None, None]
        ii = np.arange(3)[None, :, None]
        jj = np.arange(48)[None, None, :]
        mask3 = (P * ii + pp <= NCORES * jj + c).astype(ml_dtypes.bfloat16)
        m = {
            "xq": np.ascontiguousarray(x2[qrows, :].T).astype(ml_dtypes.bfloat16),
            "xkv": np.ascontiguousarray(x2[kvrows, :].T).astype(
                ml_dtypes.bfloat16
            ),
            "cosq": cosq,
            "sinq": sinq,
            "cosk": cosk,
            "sink": sink,
            "mask3": mask3,
        }
        m.update(weights)
        in_maps.append(m)
    return in_maps


def kernel(x, position_ids, Wq, Wk, Wv, Wo):
    if "nc" not in _cache:
        _cache["nc"] = _build()
    nc = _cache["nc"]
    in_maps = _host_prep(x, position_ids, Wq, Wk, Wv, Wo)
    res = bass_utils.run_bass_kernel_spmd(
        nc, in_maps, core_ids=list(range(NCORES))
    )
    out = np.empty((1, S, D), dtype=np.float32)
    for c in range(NCORES):
        outT = res.results[c]["out"].reshape(D, SL)  # [768, 512]
        out[0, c::NCORES, :] = outT.T
    return out



# revision 8
# speedup vs baseline: 1.1952x; 1.1952x over previous
"""Causal self-attention (B=1, S=4096, D=768, H=12, dh=64) on 8 TRN2 NeuronCores.

Strategy:
  - Sequence-parallel QKV projections + RoPE (each core projects 512 rows).
  - K/V (bf16; V carries a ones-column for the softmax denominator) are
    projected, rope'd and bounced out one 128-row quarter at a time, each
    quarter feeding its own AllGather so gathers overlap later projection
    work; the Q projection is emitted last to overlap the gathers too.
  - Attention is query-sharded with a stride-8 interleave (core c owns query
    rows c::8) so causal work is balanced and the program is SPMD-uniform;
    all per-core variation is input data (x slices, rope tables, masks).
  - KV rows are owned in interleaved 128-row chunks (chunk k -> core k%8) so the
    gather splits into 4 pipelined quarter-AllGathers, each delivering key
    chunks 8u..8u+7 in causal consumption order; each core's piece is a whole
    chunk, so the gathered slab reloads into SBUF with one big
    contiguous-descriptor DMA per quarter (per K and per V).
  - Transposed layout throughout: S^T = K^T.T @ Q^T has keys on partitions, so
    the softmax sum falls out of the AV matmul via the ones-row of V.
  - QK runs two heads concurrently via PE row-groups (0,*)/(64,*) with separate
    PSUM banks (3-chunk groups: 2x3 st banks + 2 ot accumulators = 8 banks).
  - Causal mask = per-128-key-chunk band multiply on a [128,3,48] window
    (band position within a group is core-independent; mask values are data).
  - RoPE: rot = A*cos + swap(A)*sin_signed with the sign in the host-built
    sin table; for K the swap is 4 quadrant-aligned DVE copies straight out
    of PSUM (keeps the AllGather-feeding chain short), for Q it is SBUF DMAs
    off a Scalar-engine PSUM copy. Logit scale is folded into exp().
"""

import numpy as np
import ml_dtypes

import concourse.bass as bass
import concourse.bacc as bacc
import concourse.tile as tile
import concourse.mybir as mybir
import concourse.bass_utils as bass_utils

NCORES = 8
S = 4096
D = 768
H = 12
DH = 64
HALF = 32
P = 128
SL = S // NCORES          # 512 local queries / kv rows per core
KSUB = D // P             # 6
NKC = S // P              # 32 key chunks of 128
GK = 4                    # key chunks per exp group
NG = NKC // GK            # 8 groups
KS = D * SL               # K^T slice elems (768*512)
VW = H * (DH + 1)         # 780: V row width incl. ones col per head
VS = SL * VW              # V slice elems
NQ = 4                    # pipelined AllGather quarters
KQ = D * P                # K^T part per quarter (768*128)
VQ = P * VW               # V part per quarter
RQ = KQ + VQ              # per-rank elems per quarter
F32 = mybir.dt.float32
BF16 = mybir.dt.bfloat16

_cache = {}


def _build(repeats=1, fake_gather=False, stop_after=None):
    nc = bacc.Bacc(
        "TRN2",
        target_bir_lowering=False,
        debug=False,
        enable_asserts=False,
        num_devices=1 if fake_gather else NCORES,
    )
    inp = {}
    for name, shape, dt in [
        ("xq", [D, SL], BF16),
        ("xkv", [D, SL], BF16),
        ("cosq", [P, SL], BF16),
        ("sinq", [P, SL], BF16),
        ("cosk", [P, SL], BF16),
        ("sink", [P, SL], BF16),
        ("mask3", [P, 3, 48], BF16),
        ("wq", [D, D], BF16),
        ("wk", [D, D], BF16),
        ("wv", [D, D], BF16),
        ("wo", [D, D], BF16),
    ]:
        inp[name] = nc.dram_tensor(name, shape, dt, kind="ExternalInput")
    out_d = nc.dram_tensor("out", [KSUB, P, SL], F32, kind="ExternalOutput")

    with tile.TileContext(nc) as tc:
      for _rep in range(repeats):
        with (
            tc.tile_pool(name="persist", bufs=1) as persist,
            tc.tile_pool(name="dram", bufs=1, space="DRAM") as dram,
        ):
            # ---- persistent tiles ----
            qrot_t = [
                persist.tile([P, SL], BF16, name=f"qrot{s_}", tag=f"qrot{s_}")
                for s_ in range(KSUB)
            ]
            osb = persist.tile([64, H, SL], BF16)
            mask_sb = persist.tile([P, 3, 48], BF16)
            nc.sync.dma_start(mask_sb[:], inp["mask3"].ap())
            wo_sb = persist.tile([64, H, D], BF16)
            nc.sync.dma_start(
                wo_sb[:], inp["wo"].ap().rearrange("(h p) e -> p h e", p=64)
            )

            kvin = dram.tile([NQ, RQ], BF16)
            kvout = [
                dram.tile(
                    [NCORES, RQ],
                    BF16,
                    name=f"kvout{u}",
                    addr_space="Local" if fake_gather else "Shared",
                )
                for u in range(NQ)
            ]

            # ================= Phase A: projections + rope =================
            with (
                tc.tile_pool(name="pw", bufs=1) as pw,
                tc.tile_pool(name="px", bufs=1) as px,
                tc.tile_pool(name="pt", bufs=3) as pt,
                tc.tile_pool(name="psA", bufs=2, space="PSUM") as psA,
            ):
                w_sb = {}
                for name in ["wq", "wk", "wv"]:
                    w_sb[name] = pw.tile([P, KSUB, D], BF16, name=f"{name}_sb")
                    nc.sync.dma_start(
                        w_sb[name][:],
                        inp[name].ap().rearrange("(ks p) m -> p ks m", p=P),
                    )
                xq_sb = px.tile([P, KSUB, SL], BF16)
                xkv_sb = px.tile([P, KSUB, SL], BF16)
                nc.sync.dma_start(
                    xq_sb[:], inp["xq"].ap().rearrange("(ks p) n -> p ks n", p=P)
                )
                nc.sync.dma_start(
                    xkv_sb[:], inp["xkv"].ap().rearrange("(ks p) n -> p ks n", p=P)
                )
                trig = {}
                for name in ["cosq", "sinq", "cosk", "sink"]:
                    trig[name] = px.tile([P, SL], BF16, name=f"{name}_sb")
                    nc.sync.dma_start(trig[name][:], inp[name].ap())

                vloc = px.tile([P, S // P // NCORES, H, DH + 1], BF16)

                def project_rope(wname, x_sb, cos_t, sin_t, dest, subs):
                    # sin_t is block-signed: rows 0-31=-sin, 32-63=+sin, etc.
                    for s in subs:
                        pa = psA.tile([P, SL], F32, name="pa", tag="pa")
                        for ks in range(KSUB):
                            nc.tensor.matmul(
                                pa[:],
                                lhsT=w_sb[wname][:, ks, s * P : (s + 1) * P],
                                rhs=x_sb[:, ks, :],
                                start=(ks == 0),
                                stop=(ks == KSUB - 1),
                            )
                        pab = pt.tile([P, SL], BF16, name="pab", tag="pab")
                        nc.scalar.copy(pab[:], pa[:])
                        swp = pt.tile([P, SL], BF16, name="swp", tag="swp")
                        for (dd, ss2) in [(0, 32), (32, 0), (64, 96), (96, 64)]:
                            nc.sync.dma_start(
                                swp[dd : dd + 32, :], pab[ss2 : ss2 + 32, :]
                            )
                        t1 = pt.tile([P, SL], BF16, name="t1", tag="t1")
                        t2 = pt.tile([P, SL], BF16, name="t2", tag="t2")
                        nc.vector.tensor_mul(t1[:], pab[:], cos_t[:])
                        nc.vector.tensor_mul(t2[:], swp[:], sin_t[:])
                        nc.vector.tensor_add(dest[s][:], t1[:], t2[:])

                # K + V projection, rope and bounce-out one QUARTER (128 kv
                # rows) at a time so each quarter-AllGather launches as soon
                # as its data exists, overlapping later projection work and
                # attention on early key chunks.
                nc.vector.memset(vloc[:, :, :, DH : DH + 1], 1.0)
                for u in range(NQ):
                    kq = pt.tile([P, KSUB, P], BF16, name="kq", tag="kq")
                    for s in range(KSUB):
                        pa = psA.tile([P, P], F32, name="pak", tag="pak")
                        for ks in range(KSUB):
                            nc.tensor.matmul(
                                pa[:],
                                lhsT=w_sb["wk"][:, ks, s * P : (s + 1) * P],
                                rhs=xkv_sb[:, ks, u * P : (u + 1) * P],
                                start=(ks == 0),
                                stop=(ks == KSUB - 1),
                            )
                        swp = pt.tile([P, P], BF16, name="swpk", tag="swpk")
                        for (dd, ss2) in [(0, 32), (32, 0), (64, 96), (96, 64)]:
                            nc.vector.tensor_copy(
                                swp[dd : dd + 32, :], pa[ss2 : ss2 + 32, :]
                            )
                        t1 = pt.tile([P, P], BF16, name="t1k", tag="t1k")
                        t2 = pt.tile([P, P], BF16, name="t2k", tag="t2k")
                        nc.vector.tensor_mul(
                            t1[:], pa[:], trig["cosk"][:, u * P : (u + 1) * P]
                        )
                        nc.vector.tensor_mul(
                            t2[:], swp[:], trig["sink"][:, u * P : (u + 1) * P]
                        )
                        nc.vector.tensor_add(kq[:, s, :], t1[:], t2[:])
                    pv = psA.tile([P, 2, SL], F32, name="pv", tag="pv")
                    for j in range(2):
                        for ks in range(KSUB):
                            nc.tensor.matmul(
                                pv[:, j, 0 : D // 2],
                                lhsT=xkv_sb[:, ks, u * P : (u + 1) * P],
                                rhs=w_sb["wv"][:, ks, j * (D // 2) : (j + 1) * (D // 2)],
                                start=(ks == 0),
                                stop=(ks == KSUB - 1),
                            )
                    for j in range(2):
                        nc.scalar.copy(
                            vloc[:, u, j * 6 : (j + 1) * 6, 0:DH],
                            pv[:, j, 0 : D // 2].rearrange("p (h d) -> p h d", d=DH),
                        )
                    nc.sync.dma_start(
                        kvin[u, 0:KQ].rearrange("(p ks n) -> p ks n", p=P, ks=KSUB),
                        kq[:],
                    )
                    nc.sync.dma_start(
                        kvin[u, KQ:].rearrange("(p h d) -> p h d", p=P, h=H),
                        vloc[:, u, :, :],
                    )
                    if fake_gather:
                        for c in range(NCORES):
                            nc.sync.dma_start(kvout[u][c], kvin[u])
                    else:
                        nc.gpsimd.collective_compute(
                            "AllGather",
                            mybir.AluOpType.bypass,
                            replica_groups=[list(range(NCORES))],
                            ins=[kvin[u].opt()],
                            outs=[kvout[u][:].opt()],
                        )
                    if u == 0:
                        # early Q subtiles: head-pair 0 can start its
                        # (ACT-bound) attention under the remaining
                        # (PE-bound) K-quarter projections
                        project_rope(
                            "wq", xq_sb, trig["cosq"], trig["sinq"], qrot_t, [0, 1]
                        )

                project_rope(
                    "wq", xq_sb, trig["cosq"], trig["sinq"], qrot_t, range(2, KSUB)
                )

            # ================= Phase B: attention =================
            if stop_after == "A":
                continue
            with (
                tc.tile_pool(name="pkv", bufs=1) as pkv,
                tc.tile_pool(name="pe", bufs=4) as pe,
                tc.tile_pool(name="pn", bufs=3) as pn,
                tc.tile_pool(name="psS", bufs=1, space="PSUM") as psS,
                tc.tile_pool(name="psO", bufs=1, space="PSUM") as psO,
            ):
                ksb_q, vsb_q = [], []
                for u in range(NQ):
                    # core c's quarter-u piece is exactly global key chunk
                    # 8u+c, so the whole gathered slab reloads as one DMA
                    # with 1536B contiguous runs on both sides.
                    kt = pkv.tile(
                        [P, NCORES, KSUB, P], BF16, name=f"ksbq{u}", tag=f"ksbq{u}"
                    )
                    nc.sync.dma_start(
                        kt[:].rearrange("p c ks n -> p c (ks n)"),
                        kvout[u][:, 0:KQ].rearrange(
                            "c (p f) -> p c f", p=P
                        ),
                    )
                    ksb_q.append(kt)
                    vt = pkv.tile(
                        [P, NCORES, H, DH + 1], BF16, name=f"vsbq{u}", tag=f"vsbq{u}"
                    )
                    nc.sync.dma_start(
                        vt[:].rearrange("p c h d -> p c (h d)"),
                        kvout[u][:, KQ:].rearrange("c (p f) -> p c f", p=P),
                    )
                    vsb_q.append(vt)

                NG3 = (NKC + 2) // 3  # 11 groups of <=3 chunks
                for hp in range(H // 2 if stop_after != "KV" else 0):
                    s = hp
                    ots = [
                        psO.tile([DH + 1, SL], F32, name=f"ot{j}", tag=f"ot{j}")
                        for j in range(2)
                    ]
                    for g in range(NG3):
                        chunks = range(3 * g, min(3 * g + 3, NKC))
                        nch = len(chunks)
                        xs = 48 * g
                        sts = [
                            psS.tile([P, 3, SL], F32, name=f"st{j}", tag=f"st{j}")
                            for j in range(2)
                        ]
                        # interleave the two heads' QK matmuls: row groups
                        # (0,*) and (64,*) run concurrently on the PE array
                        for i, kc in enumerate(chunks):
                            for j in range(2):
                                off = 64 * j
                                nc.tensor.matmul(
                                    sts[j][:, i, xs:SL],
                                    lhsT=ksb_q[kc // 8][off : off + 64, kc % 8, s, :],
                                    rhs=qrot_t[s][off : off + 64, xs:SL],
                                    start=True,
                                    stop=True,
                                )
                        expss = []
                        for j in range(2):
                            exps = pe.tile(
                                [P, 3, SL], BF16, name=f"exps{j}", tag=f"exps{j}"
                            )
                            nc.scalar.activation(
                                exps[:, 0:nch, xs:SL],
                                sts[j][:, 0:nch, xs:SL],
                                mybir.ActivationFunctionType.Exp,
                                scale=0.125,
                            )
                            mw = min(48, SL - xs)
                            nc.vector.tensor_mul(
                                exps[:, 0:nch, xs : xs + mw],
                                exps[:, 0:nch, xs : xs + mw],
                                mask_sb[:, 0:nch, 0:mw],
                            )
                            expss.append(exps)
                        for i, kc in enumerate(chunks):
                            for j in range(2):
                                nc.tensor.matmul(
                                    ots[j][:, xs:SL],
                                    lhsT=vsb_q[kc // 8][:, kc % 8, 2 * hp + j, :],
                                    rhs=expss[j][:, i, xs:SL],
                                    start=(kc == 0),
                                    stop=(kc == NKC - 1),
                                    skip_group_check=True,
                                )
                    for j in range(2):
                        h = 2 * hp + j
                        ot = ots[j]
                        # partition 64 is quadrant-aligned: the DVE can move
                        # the denominator row straight to partition 0 (probed
                        # on HW), shortening the normalize chain by a DMA hop.
                        den = pn.tile([1, SL], F32, name="den", tag="den")
                        nc.vector.tensor_copy(den[0:1, :], ot[64:65, :])
                        recip = pn.tile([1, SL], F32, name="recip", tag="recip")
                        nc.vector.reciprocal(recip[:], den[:])
                        recipb = pn.tile([64, SL], F32, name="recipb", tag="recipb")
                        nc.gpsimd.partition_broadcast(recipb[:], recip[:])
                        nc.vector.tensor_mul(osb[:, h, :], ot[0:64, :], recipb[:])

            # ================= Phase C: output projection =================
            if stop_after in ("B", "KV"):
                continue
            with (
                tc.tile_pool(name="pco", bufs=2) as pco,
                tc.tile_pool(name="psC", bufs=2, space="PSUM") as psC,
            ):
                for m in range(KSUB):
                    outp = psC.tile([P, SL], F32, name="outp", tag="outp")
                    for h in range(H):
                        nc.tensor.matmul(
                            outp[:],
                            lhsT=wo_sb[:, h, m * P : (m + 1) * P],
                            rhs=osb[:, h, :],
                            start=(h == 0),
                            stop=(h == H - 1),
                        )
                    ocp = pco.tile([P, SL], F32, name="ocp", tag="ocp")
                    nc.any.tensor_copy(ocp[:], outp[:])
                    nc.sync.dma_start(out_d.ap()[m], ocp[:])

    nc.compile()
    return nc


def _host_prep(x, position_ids, Wq, Wk, Wv, Wo):
    x2 = np.asarray(x, dtype=np.float32).reshape(S, D)
    pos = np.asarray(position_ids).reshape(S)

    fraction = (2.0 * np.arange(HALF, dtype=np.float32) / DH).astype(np.float32)
    timescale = (10000.0 ** fraction).astype(np.float32)  # [32]

    def tables(p_vec):
        sinu = (p_vec[None, :].astype(np.float32) / timescale[:, None]).astype(
            np.float32
        )
        cos = np.tile(np.cos(sinu).astype(np.float32), (4, 1))
        sin = np.sin(sinu).astype(np.float32)
        # signed for the swap formulation: first-half rows get -sin (they
        # subtract the swapped second half), second-half rows get +sin.
        sin = np.concatenate([-sin, sin, -sin, sin], axis=0)
        return cos.astype(ml_dtypes.bfloat16), sin.astype(ml_dtypes.bfloat16)

    bf = ml_dtypes.bfloat16
    weights = {
        "wq": np.ascontiguousarray(np.asarray(Wq, dtype=np.float32)).astype(bf),
        "wk": np.ascontiguousarray(np.asarray(Wk, dtype=np.float32)).astype(bf),
        "wv": np.ascontiguousarray(np.asarray(Wv, dtype=np.float32)).astype(bf),
        "wo": np.ascontiguousarray(np.asarray(Wo, dtype=np.float32)).astype(bf),
    }

    in_maps = []
    for c in range(NCORES):
        qrows = np.arange(SL) * NCORES + c
        # kv rows: 128-row key chunks k with k % 8 == c, in ascending order
        kvrows = (
            (np.arange(NQ) * NCORES + c)[:, None] * P + np.arange(P)[None, :]
        ).ravel()
        cosq, sinq = tables(pos[qrows])
        cosk, sink = tables(pos[kvrows])
        pp = np.arange(P)[:, None, None]
        ii = np.arange(3)[None, :, None]
        jj = np.arange(48)[None, None, :]
        mask3 = (P * ii + pp <= NCORES * jj + c).astype(ml_dtypes.bfloat16)
        m = {
            "xq": np.ascontiguousarray(x2[qrows, :].T).astype(ml_dtypes.bfloat16),
            "xkv": np.ascontiguousarray(x2[kvrows, :].T).astype(
                ml_dtypes.bfloat16
            ),
            "cosq": cosq,
            "sinq": sinq,
            "cosk": cosk,
            "sink": sink,
            "mask3": mask3,
        }
        m.update(weights)
        in_maps.append(m)
    return in_maps


def kernel(x, position_ids, Wq, Wk, Wv, Wo):
    if "nc" not in _cache:
        _cache["nc"] = _build()
    nc = _cache["nc"]
    in_maps = _host_prep(x, position_ids, Wq, Wk, Wv, Wo)
    res = bass_utils.run_bass_kernel_spmd(
        nc, in_maps, core_ids=list(range(NCORES))
    )
    out = np.empty((1, S, D), dtype=np.float32)
    for c in range(NCORES):
        outT = res.results[c]["out"].reshape(D, SL)  # [768, 512]
        out[0, c::NCORES, :] = outT.T
    return out



# revision 18
# speedup vs baseline: 1.2718x; 1.0641x over previous
"""Causal self-attention (B=1, S=4096, D=768, H=12, dh=64) on 8 TRN2 NeuronCores.

Strategy:
  - Sequence-parallel QKV projections + RoPE (each core projects 512 rows).
  - KV rows owned in interleaved 128-row key chunks (chunk k -> core k%8), so
    each quarter-AllGather delivers key chunks 8u..8u+7 in causal consumption
    order and each core's piece is a whole chunk: the gathered slab reloads
    into SBUF with one big contiguous-descriptor DMA per quarter per K/V,
    issued inside the projection loop so reloads pipeline behind gathers.
  - Attention is query-sharded with a stride-8 interleave (core c owns query
    rows c::8) and iterated ROUND-major (key quarter u outer, head pair
    inner): early rounds run while later quarters are still gathering.
    Per-round PSUM partials are drained to SBUF f32 accumulators on the DVE.
  - QK^T is computed transposed (keys on PSUM partitions); exp on the scalar
    engine (one instr per chunk covering both heads, window [16*kc:512]);
    causal band mask = one tiny [128,2,16] multiply on the DVE.
  - AV uses the QUERY-partition formulation: out[q,65] += exps[k,q]^T V[k,65]
    costs 65 PE cycles per chunk/qblock (output free size), 2-4x cheaper than
    streaming the query window. V carries a ones column so out[:,64] is the
    softmax denominator: normalize = per-partition reciprocal+scalar-mul.
  - O^T for the output projection comes from [128,128] DMA-transposes
    (both heads of a pair side by side), making phase C a 128-deep
    contraction over 6 head pairs.
  - RoPE: rot = A*cos + swap(A)*sin_signed computed with partition-offset
    quadrant multiplies straight out of PSUM (sign baked into the host sin
    table); no swap copies or DMAs.
"""

import numpy as np
import ml_dtypes

import concourse.bass as bass
import concourse.bacc as bacc
import concourse.tile as tile
import concourse.mybir as mybir
import concourse.bass_utils as bass_utils

NCORES = 8
S = 4096
D = 768
H = 12
DH = 64
HALF = 32
P = 128
SL = S // NCORES          # 512 local queries / kv rows per core
KSUB = D // P             # 6
NKC = S // P              # 32 key chunks of 128
NHP = H // 2              # 6 head pairs
KS = D * SL
VW = H * (DH + 1)         # 780: V row width incl. ones col per head
NQ = 4                    # pipelined AllGather quarters
KQ = D * P                # K^T part per quarter (768*128)
VQ = P * VW               # V part per quarter
RQ = KQ + VQ              # per-rank elems per quarter
F32 = mybir.dt.float32
BF16 = mybir.dt.bfloat16

_cache = {}

# rope quadrant swap: dest rows dd:dd+32 read source rows ss:ss+32
SWAP = [(0, 32), (32, 0), (64, 96), (96, 64)]


def _build(repeats=1, fake_gather=False, stop_after=None):
    nc = bacc.Bacc(
        "TRN2",
        target_bir_lowering=False,
        debug=False,
        enable_asserts=False,
        num_devices=1 if fake_gather else NCORES,
    )
    inp = {}
    for name, shape, dt in [
        ("xq", [D, SL], BF16),
        ("xkv", [D, SL], BF16),
        ("cosq", [P, SL], BF16),
        ("sinq", [P, SL], BF16),
        ("cosk", [P, SL], BF16),
        ("sink", [P, SL], BF16),
        ("mask2", [P, 2, 16], BF16),
        ("wq", [D, D], BF16),
        ("wk", [D, D], BF16),
        ("wv", [D, D], BF16),
        ("wo", [D, D], BF16),
    ]:
        inp[name] = nc.dram_tensor(name, shape, dt, kind="ExternalInput")
    out_d = nc.dram_tensor("out", [KSUB, P, SL], F32, kind="ExternalOutput")

    with tile.TileContext(nc) as tc:
      for _rep in range(repeats):
        with (
            tc.tile_pool(name="persist", bufs=1) as persist,
            tc.tile_pool(name="dram", bufs=1, space="DRAM") as dram,
        ):
            # ---- persistent tiles ----
            qrot_t = [
                persist.tile([P, SL], BF16, name=f"qrot{s_}", tag=f"qrot{s_}")
                for s_ in range(KSUB)
            ]
            mask_sb = persist.tile([P, 2, 16], BF16)
            nc.sync.dma_start(mask_sb[:], inp["mask2"].ap())
            # gathered K/V per quarter
            ksb_q = [
                persist.tile(
                    [P, NCORES, KSUB, P], BF16, name=f"ksbq{u}", tag=f"ksbq{u}"
                )
                for u in range(NQ)
            ]
            vsb_q = [
                persist.tile(
                    [P, NCORES, H, DH + 1], BF16, name=f"vsbq{u}", tag=f"vsbq{u}"
                )
                for u in range(NQ)
            ]
            # per-pair unnormalized O accumulators (q-partition layout):
            # [q, j, qblock, dh+den]
            osum = [
                persist.tile(
                    [P, 2, 4, DH + 1], F32, name=f"osum{hp}", tag=f"osum{hp}"
                )
                for hp in range(NHP)
            ]
            # O^T staging for phase C: partitions = (j*64+d) of pair hp
            osb2 = persist.tile([P, NHP, 4, P], BF16)
            wo_sb = persist.tile([P, NHP, D], BF16)

            kvin = dram.tile([NQ, RQ], BF16)
            kvout = [
                dram.tile(
                    [NCORES, RQ],
                    BF16,
                    name=f"kvout{u}",
                    addr_space="Local" if fake_gather else "Shared",
                )
                for u in range(NQ)
            ]

            # ================= Phase A: projections + rope =================
            with (
                tc.tile_pool(name="pw", bufs=1) as pw,
                tc.tile_pool(name="px", bufs=1) as px,
                tc.tile_pool(name="pt", bufs=3) as pt,
                tc.tile_pool(name="psA", bufs=2, space="PSUM") as psA,
            ):
                w_sb = {}
                xq_sb = px.tile([P, KSUB, SL], BF16)
                xkv_sb = px.tile([P, KSUB, SL], BF16)
                trig = {}
                nc.sync.dma_start(
                    xkv_sb[:], inp["xkv"].ap().rearrange("(ks p) n -> p ks n", p=P)
                )
                for name in ["wk", "wv", "wq"]:
                    w_sb[name] = pw.tile([P, KSUB, D], BF16, name=f"{name}_sb")
                    nc.sync.dma_start(
                        w_sb[name][:],
                        inp[name].ap().rearrange("(ks p) m -> p ks m", p=P),
                    )
                for name in ["cosk", "sink", "cosq", "sinq"]:
                    trig[name] = px.tile([P, SL], BF16, name=f"{name}_sb")
                    nc.sync.dma_start(trig[name][:], inp[name].ap())
                nc.sync.dma_start(
                    xq_sb[:], inp["xq"].ap().rearrange("(ks p) n -> p ks n", p=P)
                )

                vloc = px.tile([P, NQ, H, DH + 1], BF16)
                nc.vector.memset(vloc[:, :, :, DH : DH + 1], 1.0)

                def rope_from(dest, src, cos_ap, sin_ap, tmp_shape):
                    # dest = src * cos + quadswap(src) * signed-sin, with the
                    # quadrant swap done by partition-offset reads of src.
                    t1 = pt.tile(tmp_shape, BF16, name="t1", tag="t1")
                    t2 = pt.tile(tmp_shape, BF16, name="t2", tag="t2")
                    nc.vector.tensor_mul(t1[:], src[:], cos_ap)
                    for (dd, ss) in SWAP:
                        nc.vector.tensor_mul(
                            t2[dd : dd + 32, :],
                            src[ss : ss + 32, :],
                            sin_ap[dd : dd + 32, :],
                        )
                    nc.vector.tensor_add(dest, t1[:], t2[:])

                # K + V projection, rope and bounce-out one QUARTER (one
                # 128-row key chunk) at a time; each quarter's AllGather and
                # SBUF reload are issued here so they pipeline behind the
                # remaining projection work.
                for u in range(NQ):
                    kq = pt.tile([P, KSUB, P], BF16, name="kq", tag="kq")
                    for s in range(KSUB):
                        pa = psA.tile([P, P], F32, name="pak", tag="pak")
                        for ks in range(KSUB):
                            nc.tensor.matmul(
                                pa[:],
                                lhsT=w_sb["wk"][:, ks, s * P : (s + 1) * P],
                                rhs=xkv_sb[:, ks, u * P : (u + 1) * P],
                                start=(ks == 0),
                                stop=(ks == KSUB - 1),
                            )
                        rope_from(
                            kq[:, s, :],
                            pa,
                            trig["cosk"][:, u * P : (u + 1) * P],
                            trig["sink"][:, u * P : (u + 1) * P],
                            [P, P],
                        )
                    pv = psA.tile([P, 2, SL], F32, name="pv", tag="pv")
                    for j in range(2):
                        for ks in range(KSUB):
                            nc.tensor.matmul(
                                pv[:, j, 0 : D // 2],
                                lhsT=xkv_sb[:, ks, u * P : (u + 1) * P],
                                rhs=w_sb["wv"][:, ks, j * (D // 2) : (j + 1) * (D // 2)],
                                start=(ks == 0),
                                stop=(ks == KSUB - 1),
                            )
                    for j in range(2):
                        nc.scalar.copy(
                            vloc[:, u, j * 6 : (j + 1) * 6, 0:DH],
                            pv[:, j, 0 : D // 2].rearrange("p (h d) -> p h d", d=DH),
                        )
                    nc.sync.dma_start(
                        kvin[u, 0:KQ].rearrange("(p ks n) -> p ks n", p=P, ks=KSUB),
                        kq[:],
                    )
                    nc.sync.dma_start(
                        kvin[u, KQ:].rearrange("(p h d) -> p h d", p=P, h=H),
                        vloc[:, u, :, :],
                    )
                    if fake_gather:
                        for c in range(NCORES):
                            nc.sync.dma_start(kvout[u][c], kvin[u])
                    else:
                        nc.gpsimd.collective_compute(
                            "AllGather",
                            mybir.AluOpType.bypass,
                            replica_groups=[list(range(NCORES))],
                            ins=[kvin[u].opt()],
                            outs=[kvout[u][:].opt()],
                        )
                    # reload the gathered slab: one DMA per K and per V with
                    # 1536B contiguous runs on both sides.
                    nc.sync.dma_start(
                        ksb_q[u][:].rearrange("p c ks n -> p c (ks n)"),
                        kvout[u][:, 0:KQ].rearrange("c (p f) -> p c f", p=P),
                    )
                    nc.sync.dma_start(
                        vsb_q[u][:].rearrange("p c h d -> p c (h d)"),
                        kvout[u][:, KQ:].rearrange("c (p f) -> p c f", p=P),
                    )

                # Q projection + rope (after KV so the gathers launch early)
                for s in range(KSUB):
                    pa = psA.tile([P, SL], F32, name="paq", tag="paq")
                    for ks in range(KSUB):
                        nc.tensor.matmul(
                            pa[:],
                            lhsT=w_sb["wq"][:, ks, s * P : (s + 1) * P],
                            rhs=xq_sb[:, ks, :],
                            start=(ks == 0),
                            stop=(ks == KSUB - 1),
                        )
                    rope_from(
                        qrot_t[s][:], pa, trig["cosq"][:], trig["sinq"][:], [P, SL]
                    )

                nc.sync.dma_start(
                    wo_sb[:], inp["wo"].ap().rearrange("(hp p) e -> p hp e", p=P)
                )

            # ================= Phase B: attention =================
            if stop_after == "A":
                continue
            with (
                tc.tile_pool(name="pe", bufs=3) as pe,
                tc.tile_pool(name="pn", bufs=2) as pn,
                tc.tile_pool(name="psS", bufs=2, space="PSUM") as psS,
                tc.tile_pool(name="psO", bufs=2, space="PSUM") as psO,
            ):
                for u in range(NQ if stop_after != "KV" else 0):
                    for hp in range(NHP):
                        # padded to 2KB so each tile is exactly one PSUM bank
                        oq = [
                            psO.tile([P, 4, P], F32, name=f"oq{j}", tag=f"oq{j}")
                            for j in range(2)
                        ]
                        sts = []
                        expss = []
                        for i in range(NCORES):
                            kc = NCORES * u + i
                            xs = 16 * kc
                            # QK^T for both heads of the pair
                            st = psS.tile([P, 2, SL], F32, name="st", tag="st")
                            for j in range(2):
                                off = 64 * j
                                nc.tensor.matmul(
                                    st[:, j, xs:SL],
                                    lhsT=ksb_q[u][off : off + 64, i, hp, :],
                                    rhs=qrot_t[hp][off : off + 64, xs:SL],
                                    start=True,
                                    stop=True,
                                )
                            sts.append(st)
                            # AV for the previous chunk (keeps PE a group
                            # ahead of the scalar engine's exp stream)
                            if i > 0:
                                _av(nc, u, hp, i - 1, sts[i - 1], expss[i - 1],
                                    oq, vsb_q)
                            exps = pe.tile([P, 2, SL], BF16, name="exps", tag="exps")
                            nc.scalar.activation(
                                exps[:, :, xs:SL],
                                st[:, :, xs:SL],
                                mybir.ActivationFunctionType.Exp,
                                scale=0.125,
                            )
                            nc.vector.tensor_mul(
                                exps[:, :, xs : xs + 16],
                                exps[:, :, xs : xs + 16],
                                mask_sb[:],
                            )
                            # stale columns below the window, read by the
                            # 32/64-aligned boundary AV piece
                            stale = (0, 16, 0, 16, 0, 16, 32, 48)[i]
                            if stale:
                                nc.vector.memset(
                                    exps[:, :, xs - stale : xs], 0.0
                                )
                            expss.append(exps)
                        _av(nc, u, hp, NCORES - 1, sts[-1], expss[-1], oq, vsb_q)
                        # drain the round's partial O into the SBUF accumulator
                        for j in range(2):
                            if u == 0:
                                nc.vector.tensor_copy(
                                    osum[hp][:, j, :, :], oq[j][:, :, 0 : DH + 1]
                                )
                            else:
                                nc.vector.tensor_add(
                                    osum[hp][:, j, u:4, :],
                                    osum[hp][:, j, u:4, :],
                                    oq[j][:, u:4, 0 : DH + 1],
                                )
                        if u == NQ - 1:
                            # normalize + transpose O^T for phase C
                            rec = pn.tile([P, 2, 4, 1], F32, name="rec", tag="rec")
                            nc.vector.reciprocal(
                                rec[:], osum[hp][:, :, :, DH : DH + 1]
                            )
                            qn = pn.tile([P, 4, 2, DH], BF16, name="qn", tag="qn")
                            for j in range(2):
                                for qb in range(4):
                                    nc.vector.tensor_scalar_mul(
                                        qn[:, qb, j, :],
                                        osum[hp][:, j, qb, 0:DH],
                                        rec[:, j, qb, :],
                                    )
                            for qb in range(4):
                                nc.sync.dma_start_transpose(
                                    osb2[:, hp, qb, :], qn[:, qb, :, :]
                                )

            # ================= Phase C: output projection =================
            if stop_after in ("B", "KV"):
                continue
            with (
                tc.tile_pool(name="pco", bufs=2) as pco,
                tc.tile_pool(name="psC", bufs=2, space="PSUM") as psC,
            ):
                for m in range(KSUB):
                    outp = psC.tile([P, SL], F32, name="outp", tag="outp")
                    for hp in range(NHP):
                        nc.tensor.matmul(
                            outp[:],
                            lhsT=wo_sb[:, hp, m * P : (m + 1) * P],
                            rhs=osb2[:, hp, :, :],
                            start=(hp == 0),
                            stop=(hp == NHP - 1),
                        )
                    ocp = pco.tile([P, SL], F32, name="ocp", tag="ocp")
                    nc.any.tensor_copy(ocp[:], outp[:])
                    nc.sync.dma_start(out_d.ap()[m], ocp[:])

    nc.compile()
    return nc


def _av(nc, u, hp, i, st, exps, oq, vsb_q):
    """AV matmuls for chunk 8u+i: out[q,65] += exps[k, qwin]^T @ V[k, 65].

    PE output-partition bases must be 32-aligned ({0,32,64,96} for <=32
    rows, {0,64} for <=64, 0 for more), so the boundary qblock starts at
    po32 = 32*(i//2) split into aligned pieces; for odd i the 16 stale exp
    columns below the true window were zeroed by the caller.
    """
    if i < 2:
        bpieces = [(0, P)]
    elif i < 4:
        bpieces = [(32, 64), (64, P)]
    else:
        bpieces = [(64, P)]
    for j in range(2):
        v = vsb_q[u][:, i, 2 * hp + j, :]
        for qb in range(u, 4):
            for (ps, pe_) in bpieces if qb == u else [(0, P)]:
                # start=True only on the bank's FIRST write: it marks the
                # whole 2KB zero-region pending-zero, so later first-touches
                # of other qblocks zero-fill without re-marking (a second
                # start=True would wipe earlier qblocks' partial sums).
                nc.tensor.matmul(
                    oq[j][ps:pe_, qb, 0 : DH + 1],
                    lhsT=exps[:, j, 128 * qb + ps : 128 * qb + pe_],
                    rhs=v,
                    start=(i == 0 and qb == u),
                    stop=(i == NCORES - 1),
                    skip_group_check=True,
                )


def _host_prep(x, position_ids, Wq, Wk, Wv, Wo):
    x2 = np.asarray(x, dtype=np.float32).reshape(S, D)
    pos = np.asarray(position_ids).reshape(S)

    fraction = (2.0 * np.arange(HALF, dtype=np.float32) / DH).astype(np.float32)
    timescale = (10000.0 ** fraction).astype(np.float32)  # [32]

    def tables(p_vec):
        sinu = (p_vec[None, :].astype(np.float32) / timescale[:, None]).astype(
            np.float32
        )
        cos = np.tile(np.cos(sinu).astype(np.float32), (4, 1))
        sin = np.sin(sinu).astype(np.float32)
        # signed for the swap formulation: first-half rows get -sin (they
        # subtract the swapped second half), second-half rows get +sin.
        sin = np.concatenate([-sin, sin, -sin, sin], axis=0)
        return cos.astype(ml_dtypes.bfloat16), sin.astype(ml_dtypes.bfloat16)

    bf = ml_dtypes.bfloat16
    weights = {
        "wq": np.ascontiguousarray(np.asarray(Wq, dtype=np.float32)).astype(bf),
        "wk": np.ascontiguousarray(np.asarray(Wk, dtype=np.float32)).astype(bf),
        "wv": np.ascontiguousarray(np.asarray(Wv, dtype=np.float32)).astype(bf),
        "wo": np.ascontiguousarray(np.asarray(Wo, dtype=np.float32)).astype(bf),
    }

    in_maps = []
    for c in range(NCORES):
        qrows = np.arange(SL) * NCORES + c
        # kv rows: 128-row key chunks k with k % 8 == c, in ascending order
        kvrows = (
            (np.arange(NQ) * NCORES + c)[:, None] * P + np.arange(P)[None, :]
        ).ravel()
        cosq, sinq = tables(pos[qrows])
        cosk, sink = tables(pos[kvrows])
        # causal band mask within a chunk's first 16 query columns:
        # key row p visible to local query window offset w iff p <= 8w + c
        pp = np.arange(P)[:, None]
        ww = np.arange(16)[None, :]
        m1 = (pp <= NCORES * ww + c).astype(ml_dtypes.bfloat16)
        mask2 = np.repeat(m1[:, None, :], 2, axis=1)
        m = {
            "xq": np.ascontiguousarray(x2[qrows, :].T).astype(ml_dtypes.bfloat16),
            "xkv": np.ascontiguousarray(x2[kvrows, :].T).astype(
                ml_dtypes.bfloat16
            ),
            "cosq": cosq,
            "sinq": sinq,
            "cosk": cosk,
            "sink": sink,
            "mask2": np.ascontiguousarray(mask2),
        }
        m.update(weights)
        in_maps.append(m)
    return in_maps


def kernel(x, position_ids, Wq, Wk, Wv, Wo):
    if "nc" not in _cache:
        _cache["nc"] = _build()
    nc = _cache["nc"]
    in_maps = _host_prep(x, position_ids, Wq, Wk, Wv, Wo)
    res = bass_utils.run_bass_kernel_spmd(
        nc, in_maps, core_ids=list(range(NCORES))
    )
    out = np.empty((1, S, D), dtype=np.float32)
    for c in range(NCORES):
        outT = res.results[c]["out"].reshape(D, SL)  # [768, 512]
        out[0, c::NCORES, :] = outT.T
    return out
